# revision 51
# baseline (speedup 1.0000x reference)
"""Causal self-attention Trainium2 kernel (B=4, T=2048, C=1024, H=16, D=64).

Sharding: 8 cores = 4 batches x 2 causally-balanced query shards.
Core c handles batch b=c//2 and the 8 interleaved query blocks
g = 2*i + (c%2), i in 0..7 (block = 128 rows).  Every core computes full-
sequence K/V for its batch plus Q for its own query rows, runs all 16 heads
of attention for those rows, and the full output projection for them, so
per-core outputs are disjoint row-slices of y: no collectives, no host math.
One program serves both query parities: the per-kb first-block mask input
(tri / zeros / ones) encodes the parity-dependent causal structure.

Device-side dataflow (matmuls fp16 in / fp32 PSUM accumulate).  All matmul
streams are >=~512 columns so LDWEIGHTS hides behind the moving-tensor
stream (a previous 65/128-col structure was weight-load bound):
  Q^T,K^T = w^T @ x^T      (w stationary; [d, t] layout, head pairs stacked)
  V       = x^T.T @ w_v    (x^T chunks stationary, w_v moving -> V in
                            natural [t, c] layout, no PE transpose)
  S^T[kb] = K[kb] @ Q^T    (K block stationary, streams the whole causally
                            valid q-tail; two heads on partition halves)
  P^T     = exp(0.125*S^T) (ACT, psum->sbuf fp16), data mask on the first
                            valid q-block per kb
  Y^T     = [V|1].T @ P^T  (V natural stationary; accumulates [65, 1024]
                            over kb; row 64 = softmax denominators)
  YT      = Y^T * bcast(1/row64)  (PE ones-matmul broadcast + DVE mul)
  Z       = YT.T @ w_out + ones x b_out  (bias as a K=1 matmul)
"""

import os

import numpy as np

B, T, C = 4, 2048, 1024
H, D = 16, 64
N_CORES = 8
P = 128
QB = 8  # local query blocks per core (of 128 rows)
KB = 16  # key blocks per sequence
PAIRS = 8  # head pairs

_COMPILED = None
LAST_EXEC_NS = None
LAST_TRACE_PATH = None

dbg_stage = os.environ.get("KERNEL_DEBUG_STAGE", "")


def _get_mybir():
    import concourse.mybir as mybir
    return mybir


def split_sync_waits(nc):
    mybir = _get_mybir()
    # This walrus build rejects instructions carrying more than one sync
    # wait (or update).  Split the extras onto NOP carriers: waits go on
    # NOPs inserted before the instruction (same engine, so they gate it),
    # updates onto NOPs after it (fire once it has completed).
    uid = [0]

    def carrier(engine, wait=None, update=None):
        uid[0] += 1
        n = mybir.InstNoOp(
            name=f"I-syncsplit-{uid[0]}",
            opcode="NoOp",
            ins=[],
            outs=[],
            sync_info=mybir.SyncInfo(
                on_wait=[wait] if wait else [],
                on_update=[update] if update else [],
            ),
        )
        n.engine = engine
        return n

    for f in nc.m.functions:
        for blk in f.blocks:
            out = []
            changed = False
            for inst in blk.instructions:
                si = inst.sync_info
                if si is None or (
                    len(si.on_wait) <= 1 and len(si.on_update) <= 1
                ):
                    out.append(inst)
                    continue
                changed = True
                waits = list(si.on_wait)
                updates = list(si.on_update)
                for w in waits[1:]:
                    out.append(carrier(inst.engine, wait=w))
                inst.sync_info = mybir.SyncInfo(
                    on_wait=waits[:1], on_update=updates[:1]
                )
                out.append(inst)
                for u in updates[1:]:
                    out.append(carrier(inst.engine, update=u))
            if changed:
                blk.instructions = out


def _build():
    import concourse.bass as bass
    import concourse.tile as tile
    import concourse.mybir as mybir
    from contextlib import ExitStack

    f32 = mybir.dt.float32
    f16 = mybir.dt.float16
    AF = mybir.ActivationFunctionType

    nc = bass.Bass(
        "TRN2", target_bir_lowering=False, debug=False, num_devices=N_CORES
    )

    def act_recip(out, in_):
        # ACT-engine reciprocal (measured max rel err ~9e-4 on [0.5, 3e4],
        # plenty for softmax normalization).  The bass wrapper refuses
        # AF.Reciprocal outright, so emit the InstActivation directly.
        eng = nc.scalar
        inputs = [eng.lower_ap(in_)]
        for arg in (0.0, 1.0, 0.0):  # bias, scale, alpha
            inputs.append(mybir.ImmediateValue(dtype=mybir.dt.float32, value=arg))
        return eng.add_instruction(
            mybir.InstActivation(
                name=nc.get_next_instruction_name(),
                func=AF.Reciprocal,
                ins=inputs,
                outs=[eng.lower_ap(out)],
            )
        )

    xT_d = nc.dram_tensor("xt", [C, T], f16, kind="ExternalInput").ap()
    xqT_d = nc.dram_tensor("xqt", [C, QB * P], f16, kind="ExternalInput").ap()
    wqkv_d = nc.dram_tensor("wqkv", [16, P, 8, P], f16, kind="ExternalInput").ap()
    wv_d = nc.dram_tensor("wv", [P, 8, 1024], f16, kind="ExternalInput").ap()
    wout_d = nc.dram_tensor("wout", [P, 8, C], f16, kind="ExternalInput").ap()
    bqkv_d = nc.dram_tensor("bqkv", [P, 16], f32, kind="ExternalInput").ap()
    bv_d = nc.dram_tensor("bv128", [P, 1024], f16, kind="ExternalInput").ap()
    bout_d = nc.dram_tensor("bout", [P, C], f16, kind="ExternalInput").ap()
    msk_d = nc.dram_tensor("msk", [P, 2, P], f16, kind="ExternalInput").ap()
    msk01_d = nc.dram_tensor("msk01", [P, 2, P], f16, kind="ExternalInput").ap()
    ones_d = nc.dram_tensor("ones1", [1, P], f16, kind="ExternalInput").ap()
    yr_d = nc.dram_tensor("yr", [P, QB, C], f16, kind="ExternalOutput").ap()
    dbg_d = {}
    if dbg_stage in ("proj", "vn", "attn"):
        dbg_d["kt"] = nc.dram_tensor(
            "kt", [P, PAIRS, T], f16, kind="ExternalOutput"
        ).ap()
        dbg_d["qt"] = nc.dram_tensor(
            "qt", [P, PAIRS, QB * P], f16, kind="ExternalOutput"
        ).ap()
    if dbg_stage in ("vn", "attn"):
        dbg_d["vn"] = nc.dram_tensor(
            "vn", [P, PAIRS, KB, 2, 65], f16, kind="ExternalOutput"
        ).ap()
    if dbg_stage == "attn":
        dbg_d["yt"] = nc.dram_tensor(
            "yt", [P, 8, QB * P], f16, kind="ExternalOutput"
        ).ap()

    with tile.TileContext(nc) as tc, ExitStack() as ctx:
        persist = ctx.enter_context(tc.tile_pool(name="persist", bufs=1))
        KT = persist.tile([P, PAIRS, T], f16)
        QT = persist.tile([P, PAIRS, QB * P], f16)
        YT = persist.tile([P, 8, QB * P], f16)
        VN = persist.tile([P, PAIRS, KB, 2, 65], f16)
        msk = persist.tile([P, 2, P], f16)
        bqs = persist.tile([P, 16], f32)
        bv = persist.tile([P, 1024], f16)
        on1 = persist.tile([1, P], f16)
        bo = persist.tile([P, C], f16)
        msk01 = persist.tile([P, 2, P], f16)

        xT_v = xT_d.rearrange("(cb p) t -> p cb t", p=P)
        xqT_v = xqT_d.rearrange("(cb p) t -> p cb t", p=P)
        wq_v = wqkv_d[0:8].rearrange("j p cb q -> p j cb q")
        wk_v = wqkv_d[8:16].rearrange("j p cb q -> p j cb q")

        # overlap mode: K head-pairs 4-7 and the V column half 1 are
        # projected as PE filler inside the (ACT-bound) attention stream
        # of pairs 0-3.  Debug stages need the full projections up front.
        overlap = dbg_stage == ""

        # XT + the pair-0 K weights are hoisted: the K projection runs FIRST
        # so the K-half pair exchange (the long-latency collective) starts as
        # early as possible; V half 0 and Q follow on the PE while the
        # exchange is in flight.
        xtpool = ctx.enter_context(tc.tile_pool(name="xt", bufs=1))
        XT = xtpool.tile([P, 8, T], f16)
        wkp0pool = ctx.enter_context(tc.tile_pool(name="wkp0", bufs=1))
        WKp0 = wkp0pool.tile([P, 1, 8, P], f16)
        nc.sync.dma_start(out=bqs, in_=bqkv_d)
        nc.sync.dma_start(out=WKp0, in_=wk_v[:, 0:1])
        # XT lands in t-quarters: the K projection consumes them in order,
        # so the first matmul starts after 1MB instead of the full 4MB.
        for t4 in range(4):
            nc.sync.dma_start(
                out=XT[:, :, 512 * t4 : 512 * (t4 + 1)],
                in_=xT_v[:, :, 512 * t4 : 512 * (t4 + 1)],
            )
        nc.vector.memset(VN[:, :, :, :, 64:65], 1.0)

        proj_pools = ExitStack()
        wvpool = proj_pools.enter_context(tc.tile_pool(name="wv", bufs=1))
        wkpool = proj_pools.enter_context(tc.tile_pool(name="wk", bufs=1))
        if True:
            WV = wvpool.tile([P, 8, 1024], f16)
            WK = wkpool.tile([P, 3, 8, P], f16)  # K weights, slot pairs 1-3
            nc.sync.dma_start(out=WK, in_=wk_v[:, 1:4])
            nc.sync.dma_start(out=msk, in_=msk_d)
            nc.sync.dma_start(out=msk01, in_=msk01_d)
            nc.sync.dma_start(out=bv, in_=bv_d)
            nc.sync.dma_start(out=on1, in_=ones_d)
            nc.sync.dma_start(out=bo, in_=bout_d)
            nc.sync.dma_start(out=WV, in_=wv_d)

            def wk_pb(pb):
                return WKp0[:, 0] if pb == 0 else WK[:, pb - 1]

            with (
                tc.tile_pool(name="psproj", bufs=3, space="PSUM") as pspool,
            ):
                # K^T projection over full T, slot pairs 0-3 (this core's
                # physical half; 4-7 arrive via the pair exchange)
                for pb in range(4):
                    for t4 in range(4):
                        ps = pspool.tile([P, 512], f32, tag="proj")
                        for cb in range(8):
                            nc.tensor.matmul(
                                ps,
                                lhsT=wk_pb(pb)[:, cb],
                                rhs=XT[:, cb, 512 * t4 : 512 * (t4 + 1)],
                                start=(cb == 0),
                                stop=(cb == 7),
                            )
                        nc.scalar.activation(
                            KT[:, pb, 512 * t4 : 512 * (t4 + 1)],
                            ps,
                            AF.Identity,
                            bias=bqs[:, 8 + pb : 9 + pb],
                        )

                # pair-wise K exchange: cores (2b, 2b+1) computed
                # complementary physical head-pair halves (host permutes the
                # weights so each core's slots 0-3 are its own half).  Stage
                # K^T slots 0-3 into both shards of a DRAM bounce;
                # ReduceScatter(add) over the pair gives every rank
                # own+partner at a fixed address; gpsimd subtracts own to
                # recover the partner's half into slots 4-7.  Everything
                # after the collective lives on the gpsimd queue so no
                # compute engine ever waits on collective latency.
                KEX_K = 4 * T
                kexpool = ctx.enter_context(
                    tc.tile_pool(name="kex", bufs=1, space="DRAM")
                )
                kex_in = kexpool.tile([2, P, KEX_K], f16)
                kex_out = kexpool.tile([P, KEX_K], f16)
                for s in range(2):
                    nc.sync.dma_start(
                        out=kex_in[s].rearrange("p (a t) -> p a t", a=4),
                        in_=KT[:, 0:4],
                    )
                nc.gpsimd.collective_compute(
                    "ReduceScatter",
                    mybir.AluOpType.add,
                    replica_groups=[[2 * i, 2 * i + 1] for i in range(4)],
                    ins=[kex_in.opt()],
                    outs=[kex_out.opt()],
                )
                nc.gpsimd.dma_start(
                    out=KT[:, 4:8],
                    in_=kex_out[:].rearrange("p (a t) -> p a t", a=4),
                )
                nc.gpsimd.tensor_sub(
                    out=KT[:, 4:8], in0=KT[:, 4:8], in1=KT[:, 0:4]
                )

                # V natural layout, both column halves (all slot pairs)
                if dbg_stage != "proj":
                    for kb in range(KB):
                        for half in range(2):
                            ps = pspool.tile([P, 512], f32, tag="proj")
                            for cb in range(8):
                                nc.tensor.matmul(
                                    ps,
                                    lhsT=XT[:, cb, kb * P : (kb + 1) * P],
                                    rhs=WV[
                                        :, cb, 512 * half : 512 * (half + 1)
                                    ],
                                    start=(cb == 0),
                                    stop=(cb == 7),
                                )
                            nc.vector.tensor_add(
                                out=VN[
                                    :, 4 * half : 4 * half + 4, kb, :, 0:64
                                ],
                                in0=ps.rearrange(
                                    "p (a b c) -> p a b c", b=2, c=64
                                ),
                                in1=bv[
                                    :, 512 * half : 512 * (half + 1)
                                ].rearrange("p (a b c) -> p a b c", b=2, c=64),
                            )

        # XT/WK/WV free here; Q projection + attention need none of them.
        proj_pools.close()

        supool = ctx.enter_context(tc.tile_pool(name="su", bufs=1))
        SU1 = supool.tile([1, 16 * QB * P], f16)  # softmax sums, part 0

        with (
            tc.tile_pool(name="xqt", bufs=1) as xqtpool,
            tc.tile_pool(name="wq", bufs=1) as wqpool,
            tc.tile_pool(name="psprojq", bufs=3, space="PSUM") as pspool,
        ):
            XQT = xqtpool.tile([P, 8, QB * P], f16)
            WQ = wqpool.tile([P, 8, 8, P], f16)
            nc.sync.dma_start(out=WQ[:, 0:1], in_=wq_v[:, 0:1])
            nc.sync.dma_start(out=XQT[:, 0:8], in_=xqT_v[:, 0:8])
            nc.sync.dma_start(out=WQ[:, 1:8], in_=wq_v[:, 1:8])
            # Q^T projection (j-blocks 0..7): w stationary, XQT moving
            for pb in range(PAIRS):
                for t4 in range(2):
                    ps = pspool.tile([P, 512], f32, tag="proj")
                    for cb in range(8):
                        nc.tensor.matmul(
                            ps,
                            lhsT=WQ[:, pb, cb],
                            rhs=XQT[:, cb, 512 * t4 : 512 * (t4 + 1)],
                            start=(cb == 0),
                            stop=(cb == 7),
                        )
                    nc.scalar.activation(
                        QT[:, pb, 512 * t4 : 512 * (t4 + 1)],
                        ps,
                        AF.Identity,
                        bias=bqs[:, pb : pb + 1],
                    )

        if True:

            if "kt" in dbg_d:
                nc.sync.dma_start(out=dbg_d["kt"], in_=KT)
                nc.sync.dma_start(out=dbg_d["qt"], in_=QT)
            if "vn" in dbg_d:
                nc.sync.dma_start(out=dbg_d["vn"], in_=VN)

            # attention: S^T pieces bin-packed two-per-[P,1024]-psum-tile
            # (one per bank-aligned 512-col compartment; a 64-contract
            # quadrant matmul with start=False hangs the device, so every
            # piece is a bank-aligned start=True).  One ACT exp covers both
            # compartments -- ACT's ~260ns fixed cost per activation
            # dominated the narrow causal tail otherwise.  Causal masking is
            # a pre-exp additive log-mask (0 / -3e4) applied by the DVE on
            # psum, hidden in the ACT queue delay.  Y^T accumulates per
            # column-half in [65, 512] psum tiles; softmax sums (row 64) are
            # parked in SU1.  In overlap mode the K pb4-7 / V half-1
            # projections interleave into pairs 0-3's ACT-bound stream as
            # PE filler.
            attn_pairs = (
                list(range(PAIRS)) if dbg_stage in ("", "attn", "full") else []
            )
            if os.environ.get("KERNEL_NPAIRS"):
                attn_pairs = attn_pairs[: int(os.environ["KERNEL_NPAIRS"])]
            DEPTH = 4  # bins in flight ahead of their Y-matmuls (4 so the
            # psY park copies drain a full bin before the next (pb, half)'s
            # first Y re-allocates the banks)
            with (
                tc.tile_pool(name="pts", bufs=DEPTH + 1) as ptpool,
                tc.tile_pool(name="psS", bufs=3, space="PSUM") as psSpool,
                tc.tile_pool(name="psY", bufs=1, space="PSUM") as psYpool,
            ):
                bins = []
                for pb in attn_pairs:
                    for half in range(2):
                        npieces = 0
                        for kb in range(KB if half else 8):
                            q0 = (kb // 2) * P
                            c0 = max(512 * half, q0)
                            c1 = 512 * (half + 1)
                            for h in range(2):
                                if npieces % 2 == 0:
                                    bins.append((pb, half, []))
                                bins[-1][2].append(
                                    (kb, h, c0, c1, q0, 512 * (npieces % 2))
                                )
                                npieces += 1

                psYT = {}

                def emit_y(bin_, pt):
                    pb, half, pieces = bin_
                    lastk = (KB - 1) if half else 7
                    for (kb, h, c0, c1, q0, off) in pieces:
                        key = (pb, half, h)
                        if key not in psYT:
                            psYT[key] = psYpool.tile(
                                [65, 512], f32, tag=f"y{h}", name=f"psYT{h}"
                            )
                        nc.tensor.matmul(
                            psYT[key][:, c0 - 512 * half : c1 - 512 * half],
                            lhsT=VN[:, pb, kb, h],
                            rhs=pt[:, off : off + c1 - c0],
                            start=(kb == 0),
                            stop=(kb == lastk),
                            skip_group_check=True,
                        )
                    if pieces[-1][0] == lastk and pieces[-1][1] == 1:
                        # (pb, half) complete for both heads: park sums +
                        # the unnormalized Y^T, freeing the psum tiles.
                        for hh in range(2):
                            ps = psYT.pop((pb, half, hh))
                            j = 2 * pb + hh
                            nc.vector.tensor_copy(
                                out=SU1[
                                    0:1,
                                    j * 1024
                                    + 512 * half : j * 1024
                                    + 512 * (half + 1),
                                ],
                                in_=ps[64:65, :],
                            )
                            nc.vector.tensor_copy(
                                out=YT[
                                    64 * hh : 64 * hh + 64,
                                    pb,
                                    512 * half : 512 * (half + 1),
                                ],
                                in_=ps[0:64, :],
                            )

                pend = []
                for bi, bin_ in enumerate(bins):
                    pb, half, pieces = bin_
                    binw = max(
                        off + c1 - c0 for (kb, h, c0, c1, q0, off) in pieces
                    )
                    psS = psSpool.tile([P, 1024], f32, tag="s", name="psS")
                    for (kb, h, c0, c1, q0, off) in pieces:
                        nc.tensor.matmul(
                            psS[:, off : off + c1 - c0],
                            lhsT=KT[
                                64 * h : 64 * h + 64, pb, kb * P : (kb + 1) * P
                            ],
                            rhs=QT[64 * h : 64 * h + 64, pb, c0:c1],
                            start=True,
                            stop=True,
                            skip_group_check=True,
                        )
                    gp_mask = pb > 0
                    if not gp_mask:
                        for (kb, h, c0, c1, q0, off) in pieces:
                            if c0 <= q0 < c1:
                                mo = off + q0 - c0
                                nc.vector.tensor_add(
                                    out=psS[:, mo : mo + P],
                                    in0=psS[:, mo : mo + P],
                                    in1=msk[:, kb % 2, :],
                                )
                    pt = ptpool.tile([P, 1024], f16, tag="pt", name="pt")
                    nc.scalar.activation(
                        pt[:, :binw], psS[:, :binw], AF.Exp, scale=0.125
                    )
                    if gp_mask:
                        for (kb, h, c0, c1, q0, off) in pieces:
                            if c0 <= q0 < c1:
                                mo = off + q0 - c0
                                nc.gpsimd.tensor_mul(
                                    out=pt[:, mo : mo + P],
                                    in0=pt[:, mo : mo + P],
                                    in1=msk01[:, kb % 2, :],
                                )
                    pend.append((bin_, pt))
                    if len(pend) > DEPTH:
                        emit_y(*pend.pop(0))
                for item in pend:
                    emit_y(*item)

        if "yt" in dbg_d:
            nc.sync.dma_start(out=dbg_d["yt"], in_=YT)
        if dbg_stage:
            nc.vector.memset(YT[:1, 0, :1], 0.0)

        # batched normalization + output projection, column-half-major so
        # the first four output blocks overlap half 1's normalization.
        # WO loads late into the space freed by XT (its DMA rides out the
        # attention tail).
        out_blocks = list(range(QB)) if dbg_stage in ("", "attn", "full") else []
        with (
            tc.tile_pool(name="wo", bufs=1) as wopool,
            tc.tile_pool(name="z", bufs=2) as zpool,
            tc.tile_pool(name="rs", bufs=3) as rspool,
            tc.tile_pool(name="psN", bufs=2, space="PSUM") as psNpool,
            tc.tile_pool(name="psZ", bufs=2, space="PSUM") as psZpool,
        ):
            WO = wopool.tile([P, 8, C], f16)
            nc.sync.dma_start(out=WO, in_=wout_d)

            def normalize(half):
                # softmax denominators: PE broadcast of the parked sums to 64
                # partitions, DVE fast reciprocal (fp32, ~51 ULP), DVE scale
                # of the unnormalized Y^T.  Keeps ACT out of the tail.
                for pb in (attn_pairs if dbg_stage in ("", "attn", "full") else []):
                    for h in range(2):
                        j = 2 * pb + h
                        rb = psNpool.tile([P, 512], f32, tag="n", name="rb")
                        nc.tensor.matmul(
                            rb[0:64, :],
                            lhsT=on1[:, 0:64],
                            rhs=SU1[
                                0:1,
                                j * 1024
                                + 512 * half : j * 1024
                                + 512 * (half + 1),
                            ],
                            start=True,
                            stop=True,
                        )
                        rsf = rspool.tile([P, 512], f16, tag="rs", name="rsf")
                        rs = rsf[64 * h : 64 * h + 64, :]
                        act_recip(rs, rb[0:64, :])
                        sl = YT[
                            64 * h : 64 * h + 64,
                            pb,
                            512 * half : 512 * (half + 1),
                        ]
                        nc.vector.tensor_mul(out=sl, in0=sl, in1=rs)

            def out_proj(iblocks):
                for i in iblocks:
                    zt = zpool.tile([P, C], f16, tag="z", name="zt")
                    ps = psZpool.tile([P, 1024], f32, tag="z", name="psZ")
                    for nc2 in range(2):
                        sub = ps[:, 512 * nc2 : 512 * (nc2 + 1)]
                        for cb in range(8):
                            nc.tensor.matmul(
                                sub,
                                lhsT=YT[:, cb, i * P : (i + 1) * P],
                                rhs=WO[:, cb, 512 * nc2 : 512 * (nc2 + 1)],
                                start=(cb == 0),
                                stop=(cb == 7),
                                skip_group_check=True,
                            )
                    # bias folded into the psum drain (bo is a host-side
                    # broadcast of b_out to 128 partitions)
                    nc.vector.tensor_add(out=zt, in0=ps, in1=bo)
                    nc.sync.dma_start(out=yr_d[:, i], in_=zt)

            if out_blocks:
                normalize(0)
                out_proj(out_blocks[0:4])
                normalize(1)
                out_proj(out_blocks[4:8])
            else:
                zt = zpool.tile([P, C], f16, tag="z", name="zt")
                nc.vector.memset(zt, 0.0)
                nc.sync.dma_start(out=yr_d[:, 0], in_=zt)

    split_sync_waits(nc)
    return nc


def _host_inputs(x, w_qkv, b_qkv, w_out, b_out):
    x = np.asarray(x, dtype=np.float32)
    w_qkv = np.asarray(w_qkv, dtype=np.float32)
    b_qkv = np.asarray(b_qkv, dtype=np.float32)
    w_out = np.asarray(w_out, dtype=np.float32)
    b_out = np.asarray(b_out, dtype=np.float32)

    wqkv_r0 = np.ascontiguousarray(
        w_qkv[:, :2048].reshape(8, P, 16, P).transpose(2, 1, 0, 3)
    ).astype(np.float16)
    wv_r0 = np.ascontiguousarray(
        w_qkv[:, 2048:].reshape(8, P, 1024).transpose(1, 0, 2)
    ).astype(np.float16)
    wout_r0 = np.ascontiguousarray(
        w_out.reshape(8, P, C).transpose(1, 0, 2)
    ).astype(np.float16)
    bqkv_r0 = np.ascontiguousarray(b_qkv[:2048].reshape(16, P).T)
    bv_r0 = np.ascontiguousarray(
        np.broadcast_to(b_qkv[2048:], (P, 1024))
    ).astype(np.float16)

    def permute_pairs(perm):
        # slot-pair s on the device maps to physical head pair perm[s]: the
        # kernel computes K/V projections for slots 0-3 only and receives
        # slots 4-7 from its partner core, so partners get complementary
        # physical halves.  All per-pair weight/bias layouts permute
        # consistently; the output projection re-contracts over all pairs, so
        # y is permutation-invariant.
        jq = list(perm)
        jk = [8 + p for p in perm]
        wqkv_r = np.ascontiguousarray(wqkv_r0[jq + jk])
        bqkv_r = np.ascontiguousarray(bqkv_r0[:, jq + jk])
        wv_r = np.ascontiguousarray(
            wv_r0.reshape(P, 8, 8, P)[:, :, perm].reshape(P, 8, 1024)
        )
        bv_r = np.ascontiguousarray(
            bv_r0.reshape(P, 8, P)[:, perm].reshape(P, 1024)
        )
        wout_r = np.ascontiguousarray(wout_r0[:, perm])
        return wqkv_r, bqkv_r, wv_r, bv_r, wout_r

    perms = [
        permute_pairs([0, 1, 2, 3, 4, 5, 6, 7]),
        permute_pairs([4, 5, 6, 7, 0, 1, 2, 3]),
    ]
    bout_r = np.ascontiguousarray(
        np.broadcast_to(b_out, (P, C))
    ).astype(np.float16)
    # additive log-masks: 0 = allowed, -3e4 = masked (exp underflows to 0)
    NEG = np.float16(-30000.0)
    tri = np.where(np.triu(np.ones((P, P), dtype=bool)), 0, NEG).astype(
        np.float16
    )  # [k, q]: k <= q allowed
    zer = np.full((P, P), NEG, dtype=np.float16)  # fully masked
    one = np.zeros((P, P), dtype=np.float16)  # fully allowed
    ones1 = np.ones((1, P), dtype=np.float16)
    tri01 = np.triu(np.ones((P, P), dtype=np.float16))  # [k, q]: k <= q
    zer01 = np.zeros((P, P), dtype=np.float16)
    one01 = np.ones((P, P), dtype=np.float16)

    in_maps = []
    for c in range(N_CORES):
        b, par = c // 2, c % 2
        wqkv_r, bqkv_r, wv_r, bv_r, wout_r = perms[par]
        xb = x[b]
        xT = np.ascontiguousarray(xb.T).astype(np.float16)
        qg = [2 * i + par for i in range(QB)]
        xq = np.concatenate([xb[g * P : (g + 1) * P] for g in qg], axis=0)
        xqT = np.ascontiguousarray(xq.T).astype(np.float16)
        # first-valid-block log-mask by kb parity: the first q block
        # i0 = kb//2 has g0 = 2*(kb//2) + par; g0 == kb -> tri,
        # g0 < kb -> fully masked, g0 > kb -> fully allowed.
        mk = np.empty((P, 2, P), dtype=np.float16)
        mk[:, 0, :] = tri if par == 0 else one
        mk[:, 1, :] = zer if par == 0 else tri
        mk01 = np.empty((P, 2, P), dtype=np.float16)
        mk01[:, 0, :] = tri01 if par == 0 else one01
        mk01[:, 1, :] = zer01 if par == 0 else tri01
        in_maps.append(
            {
                "xt": xT,
                "xqt": xqT,
                "wqkv": wqkv_r,
                "wv": wv_r,
                "wout": wout_r,
                "bqkv": bqkv_r,
                "bv128": bv_r,
                "bout": bout_r,
                "msk": mk,
                "msk01": mk01,
                "ones1": ones1,
            }
        )
    return in_maps


def kernel(x, w_qkv, b_qkv, w_out, b_out, trace=False):
    global _COMPILED, LAST_EXEC_NS, LAST_TRACE_PATH
    from concourse import bass_utils

    if _COMPILED is None:
        _COMPILED = _build()
    nc = _COMPILED

    in_maps = _host_inputs(x, w_qkv, b_qkv, w_out, b_out)
    res = bass_utils.run_bass_kernel_spmd(
        nc, in_maps, core_ids=list(range(N_CORES)), trace=trace
    )
    LAST_EXEC_NS = res.exec_time_ns
    if res.instructions_and_trace:
        LAST_TRACE_PATH = res.instructions_and_trace[1]

    y = np.empty((B, T, C), dtype=np.float32)
    for c in range(N_CORES):
        b, par = c // 2, c % 2
        yl = res.results[c]["yr"].transpose(1, 0, 2)  # [QB, P, C]
        for i in range(QB):
            g = 2 * i + par
            y[b, g * P : (g + 1) * P] = yl[i]
    return y



# revision 52
# speedup vs baseline: 1.0261x; 1.0261x over previous
"""Causal self-attention Trainium2 kernel (B=4, T=2048, C=1024, H=16, D=64).

Sharding: 8 cores = 4 batches x 2 causally-balanced query shards.
Core c handles batch b=c//2 and the 8 interleaved query blocks
g = 2*i + (c%2), i in 0..7 (block = 128 rows).  Every core computes full-
sequence K/V for its batch plus Q for its own query rows, runs all 16 heads
of attention for those rows, and the full output projection for them, so
per-core outputs are disjoint row-slices of y: no collectives, no host math.
One program serves both query parities: the per-kb first-block mask input
(tri / zeros / ones) encodes the parity-dependent causal structure.

Device-side dataflow (matmuls fp16 in / fp32 PSUM accumulate).  All matmul
streams are >=~512 columns so LDWEIGHTS hides behind the moving-tensor
stream (a previous 65/128-col structure was weight-load bound):
  Q^T,K^T = w^T @ x^T      (w stationary; [d, t] layout, head pairs stacked)
  V       = x^T.T @ w_v    (x^T chunks stationary, w_v moving -> V in
                            natural [t, c] layout, no PE transpose)
  S^T[kb] = K[kb] @ Q^T    (K block stationary, streams the whole causally
                            valid q-tail; two heads on partition halves)
  P^T     = exp(0.125*S^T) (ACT, psum->sbuf fp16), data mask on the first
                            valid q-block per kb
  Y^T     = [V|1].T @ P^T  (V natural stationary; accumulates [65, 1024]
                            over kb; row 64 = softmax denominators)
  YT      = Y^T * bcast(1/row64)  (PE ones-matmul broadcast + DVE mul)
  Z       = YT.T @ w_out + ones x b_out  (bias as a K=1 matmul)
"""

import os

import numpy as np

B, T, C = 4, 2048, 1024
H, D = 16, 64
N_CORES = 8
P = 128
QB = 8  # local query blocks per core (of 128 rows)
KB = 16  # key blocks per sequence
PAIRS = 8  # head pairs

_COMPILED = None
LAST_EXEC_NS = None
LAST_TRACE_PATH = None

dbg_stage = os.environ.get("KERNEL_DEBUG_STAGE", "")


def _get_mybir():
    import concourse.mybir as mybir
    return mybir


def split_sync_waits(nc):
    mybir = _get_mybir()
    # This walrus build rejects instructions carrying more than one sync
    # wait (or update).  Split the extras onto NOP carriers: waits go on
    # NOPs inserted before the instruction (same engine, so they gate it),
    # updates onto NOPs after it (fire once it has completed).
    uid = [0]

    def carrier(engine, wait=None, update=None):
        uid[0] += 1
        n = mybir.InstNoOp(
            name=f"I-syncsplit-{uid[0]}",
            opcode="NoOp",
            ins=[],
            outs=[],
            sync_info=mybir.SyncInfo(
                on_wait=[wait] if wait else [],
                on_update=[update] if update else [],
            ),
        )
        n.engine = engine
        return n

    for f in nc.m.functions:
        for blk in f.blocks:
            out = []
            changed = False
            for inst in blk.instructions:
                si = inst.sync_info
                if si is None or (
                    len(si.on_wait) <= 1 and len(si.on_update) <= 1
                ):
                    out.append(inst)
                    continue
                changed = True
                waits = list(si.on_wait)
                updates = list(si.on_update)
                for w in waits[1:]:
                    out.append(carrier(inst.engine, wait=w))
                inst.sync_info = mybir.SyncInfo(
                    on_wait=waits[:1], on_update=updates[:1]
                )
                out.append(inst)
                for u in updates[1:]:
                    out.append(carrier(inst.engine, update=u))
            if changed:
                blk.instructions = out


def _build():
    import concourse.bass as bass
    import concourse.tile as tile
    import concourse.mybir as mybir
    from contextlib import ExitStack

    f32 = mybir.dt.float32
    f16 = mybir.dt.float16
    AF = mybir.ActivationFunctionType

    nc = bass.Bass(
        "TRN2", target_bir_lowering=False, debug=False, num_devices=N_CORES
    )

    def act_recip(out, in_):
        # ACT-engine reciprocal (measured max rel err ~9e-4 on [0.5, 3e4],
        # plenty for softmax normalization).  The bass wrapper refuses
        # AF.Reciprocal outright, so emit the InstActivation directly.
        eng = nc.scalar
        inputs = [eng.lower_ap(in_)]
        for arg in (0.0, 1.0, 0.0):  # bias, scale, alpha
            inputs.append(mybir.ImmediateValue(dtype=mybir.dt.float32, value=arg))
        return eng.add_instruction(
            mybir.InstActivation(
                name=nc.get_next_instruction_name(),
                func=AF.Reciprocal,
                ins=inputs,
                outs=[eng.lower_ap(out)],
            )
        )

    xT_d = nc.dram_tensor("xt", [C, T], f16, kind="ExternalInput").ap()
    xqT_d = nc.dram_tensor("xqt", [C, QB * P], f16, kind="ExternalInput").ap()
    wqkv_d = nc.dram_tensor("wqkv", [16, P, 8, P], f16, kind="ExternalInput").ap()
    wv_d = nc.dram_tensor("wv", [P, 8, 1024], f16, kind="ExternalInput").ap()
    wout_d = nc.dram_tensor("wout", [P, 8, C], f16, kind="ExternalInput").ap()
    bqkv_d = nc.dram_tensor("bqkv", [P, 16], f32, kind="ExternalInput").ap()
    bv_d = nc.dram_tensor("bv128", [P, 1024], f16, kind="ExternalInput").ap()
    bout_d = nc.dram_tensor("bout", [P, C], f16, kind="ExternalInput").ap()
    msk_d = nc.dram_tensor("msk", [P, 2, P], f16, kind="ExternalInput").ap()
    ones_d = nc.dram_tensor("ones1", [1, P], f16, kind="ExternalInput").ap()
    yr_d = nc.dram_tensor("yr", [P, QB, C], f16, kind="ExternalOutput").ap()
    dbg_d = {}
    if dbg_stage in ("proj", "vn", "attn"):
        dbg_d["kt"] = nc.dram_tensor(
            "kt", [P, PAIRS, T], f16, kind="ExternalOutput"
        ).ap()
        dbg_d["qt"] = nc.dram_tensor(
            "qt", [P, PAIRS, QB * P], f16, kind="ExternalOutput"
        ).ap()
    if dbg_stage in ("vn", "attn"):
        dbg_d["vn"] = nc.dram_tensor(
            "vn", [P, PAIRS, KB, 2, 65], f16, kind="ExternalOutput"
        ).ap()
    if dbg_stage == "attn":
        dbg_d["yt"] = nc.dram_tensor(
            "yt", [P, 8, QB * P], f16, kind="ExternalOutput"
        ).ap()

    with tile.TileContext(nc) as tc, ExitStack() as ctx:
        persist = ctx.enter_context(tc.tile_pool(name="persist", bufs=1))
        KT = persist.tile([P, PAIRS, T], f16)
        QT = persist.tile([P, PAIRS, QB * P], f16)
        YT = persist.tile([P, 8, QB * P], f16)
        VN = persist.tile([P, PAIRS, KB, 2, 65], f16)
        msk = persist.tile([P, 2, P], f16)
        bqs = persist.tile([P, 16], f32)
        bv = persist.tile([P, 1024], f16)
        on1 = persist.tile([1, P], f16)
        bo = persist.tile([P, C], f16)

        xT_v = xT_d.rearrange("(cb p) t -> p cb t", p=P)
        xqT_v = xqT_d.rearrange("(cb p) t -> p cb t", p=P)
        wq_v = wqkv_d[0:8].rearrange("j p cb q -> p j cb q")
        wk_v = wqkv_d[8:16].rearrange("j p cb q -> p j cb q")

        # overlap mode: K head-pairs 4-7 and the V column half 1 are
        # projected as PE filler inside the (ACT-bound) attention stream
        # of pairs 0-3.  Debug stages need the full projections up front.
        overlap = dbg_stage == ""

        # XT + the pair-0 K weights are hoisted: the K projection runs FIRST
        # so the K-half pair exchange (the long-latency collective) starts as
        # early as possible; V half 0 and Q follow on the PE while the
        # exchange is in flight.
        xtpool = ctx.enter_context(tc.tile_pool(name="xt", bufs=1))
        XT = xtpool.tile([P, 8, T], f16)
        wkp0pool = ctx.enter_context(tc.tile_pool(name="wkp0", bufs=1))
        WKp0 = wkp0pool.tile([P, 1, 8, P], f16)
        nc.sync.dma_start(out=bqs, in_=bqkv_d)
        nc.sync.dma_start(out=WKp0, in_=wk_v[:, 0:1])
        # XT lands in t-quarters: the K projection consumes them in order,
        # so the first matmul starts after 1MB instead of the full 4MB.
        for t4 in range(4):
            nc.sync.dma_start(
                out=XT[:, :, 512 * t4 : 512 * (t4 + 1)],
                in_=xT_v[:, :, 512 * t4 : 512 * (t4 + 1)],
            )
        nc.vector.memset(VN[:, :, :, :, 64:65], 1.0)

        proj_pools = ExitStack()
        wvpool = proj_pools.enter_context(tc.tile_pool(name="wv", bufs=1))
        wkpool = proj_pools.enter_context(tc.tile_pool(name="wk", bufs=1))
        if True:
            WV = wvpool.tile([P, 8, 1024], f16)
            WK = wkpool.tile([P, 3, 8, P], f16)  # K weights, slot pairs 1-3
            nc.sync.dma_start(out=WK, in_=wk_v[:, 1:4])
            nc.sync.dma_start(out=msk, in_=msk_d)
            nc.sync.dma_start(out=bv, in_=bv_d)
            nc.sync.dma_start(out=on1, in_=ones_d)
            nc.sync.dma_start(out=bo, in_=bout_d)
            nc.sync.dma_start(out=WV, in_=wv_d)

            def wk_pb(pb):
                return WKp0[:, 0] if pb == 0 else WK[:, pb - 1]

            with (
                tc.tile_pool(name="psproj", bufs=3, space="PSUM") as pspool,
            ):
                # K^T projection over full T, slot pairs 0-3 (this core's
                # physical half; 4-7 arrive via the pair exchange)
                for pb in range(4):
                    for t4 in range(4):
                        ps = pspool.tile([P, 512], f32, tag="proj")
                        for cb in range(8):
                            nc.tensor.matmul(
                                ps,
                                lhsT=wk_pb(pb)[:, cb],
                                rhs=XT[:, cb, 512 * t4 : 512 * (t4 + 1)],
                                start=(cb == 0),
                                stop=(cb == 7),
                            )
                        nc.scalar.activation(
                            KT[:, pb, 512 * t4 : 512 * (t4 + 1)],
                            ps,
                            AF.Identity,
                            bias=bqs[:, 8 + pb : 9 + pb],
                        )

                # pair-wise K exchange: cores (2b, 2b+1) computed
                # complementary physical head-pair halves (host permutes the
                # weights so each core's slots 0-3 are its own half).  Stage
                # K^T slots 0-3 into both shards of a DRAM bounce;
                # ReduceScatter(add) over the pair gives every rank
                # own+partner at a fixed address; gpsimd subtracts own to
                # recover the partner's half into slots 4-7.  Everything
                # after the collective lives on the gpsimd queue so no
                # compute engine ever waits on collective latency.
                KEX_K = 4 * T
                kexpool = ctx.enter_context(
                    tc.tile_pool(name="kex", bufs=1, space="DRAM")
                )
                kex_in = kexpool.tile([2, P, KEX_K], f16)
                kex_out = kexpool.tile([P, KEX_K], f16)
                for s in range(2):
                    nc.sync.dma_start(
                        out=kex_in[s].rearrange("p (a t) -> p a t", a=4),
                        in_=KT[:, 0:4],
                    )
                nc.gpsimd.collective_compute(
                    "ReduceScatter",
                    mybir.AluOpType.add,
                    replica_groups=[[2 * i, 2 * i + 1] for i in range(4)],
                    ins=[kex_in.opt()],
                    outs=[kex_out.opt()],
                )
                nc.gpsimd.dma_start(
                    out=KT[:, 4:8],
                    in_=kex_out[:].rearrange("p (a t) -> p a t", a=4),
                )
                nc.gpsimd.tensor_sub(
                    out=KT[:, 4:8], in0=KT[:, 4:8], in1=KT[:, 0:4]
                )

                # V natural layout, both column halves (all slot pairs)
                if dbg_stage != "proj":
                    for kb in range(KB):
                        for half in range(2):
                            ps = pspool.tile([P, 512], f32, tag="proj")
                            for cb in range(8):
                                nc.tensor.matmul(
                                    ps,
                                    lhsT=XT[:, cb, kb * P : (kb + 1) * P],
                                    rhs=WV[
                                        :, cb, 512 * half : 512 * (half + 1)
                                    ],
                                    start=(cb == 0),
                                    stop=(cb == 7),
                                )
                            nc.vector.tensor_add(
                                out=VN[
                                    :, 4 * half : 4 * half + 4, kb, :, 0:64
                                ],
                                in0=ps.rearrange(
                                    "p (a b c) -> p a b c", b=2, c=64
                                ),
                                in1=bv[
                                    :, 512 * half : 512 * (half + 1)
                                ].rearrange("p (a b c) -> p a b c", b=2, c=64),
                            )

        # XT/WK/WV free here; Q projection + attention need none of them.
        proj_pools.close()

        supool = ctx.enter_context(tc.tile_pool(name="su", bufs=1))
        SU1 = supool.tile([1, 16 * QB * P], f16)  # softmax sums, part 0

        with (
            tc.tile_pool(name="xqt", bufs=1) as xqtpool,
            tc.tile_pool(name="wq", bufs=1) as wqpool,
            tc.tile_pool(name="psprojq", bufs=3, space="PSUM") as pspool,
        ):
            XQT = xqtpool.tile([P, 8, QB * P], f16)
            WQ = wqpool.tile([P, 8, 8, P], f16)
            nc.sync.dma_start(out=WQ[:, 0:1], in_=wq_v[:, 0:1])
            nc.sync.dma_start(out=XQT[:, 0:8], in_=xqT_v[:, 0:8])
            nc.sync.dma_start(out=WQ[:, 1:8], in_=wq_v[:, 1:8])
            # Q^T projection (j-blocks 0..7): w stationary, XQT moving
            for pb in range(PAIRS):
                for t4 in range(2):
                    ps = pspool.tile([P, 512], f32, tag="proj")
                    for cb in range(8):
                        nc.tensor.matmul(
                            ps,
                            lhsT=WQ[:, pb, cb],
                            rhs=XQT[:, cb, 512 * t4 : 512 * (t4 + 1)],
                            start=(cb == 0),
                            stop=(cb == 7),
                        )
                    nc.scalar.activation(
                        QT[:, pb, 512 * t4 : 512 * (t4 + 1)],
                        ps,
                        AF.Identity,
                        bias=bqs[:, pb : pb + 1],
                    )

        if True:

            if "kt" in dbg_d:
                nc.sync.dma_start(out=dbg_d["kt"], in_=KT)
                nc.sync.dma_start(out=dbg_d["qt"], in_=QT)
            if "vn" in dbg_d:
                nc.sync.dma_start(out=dbg_d["vn"], in_=VN)

            # attention: S^T pieces bin-packed two-per-[P,1024]-psum-tile
            # (one per bank-aligned 512-col compartment; a 64-contract
            # quadrant matmul with start=False hangs the device, so every
            # piece is a bank-aligned start=True).  One ACT exp covers both
            # compartments -- ACT's ~260ns fixed cost per activation
            # dominated the narrow causal tail otherwise.  Causal masking is
            # a pre-exp additive log-mask (0 / -3e4) applied by the DVE on
            # psum, hidden in the ACT queue delay.  Y^T accumulates per
            # column-half in [65, 512] psum tiles; softmax sums (row 64) are
            # parked in SU1.  In overlap mode the K pb4-7 / V half-1
            # projections interleave into pairs 0-3's ACT-bound stream as
            # PE filler.
            attn_pairs = (
                list(range(PAIRS)) if dbg_stage in ("", "attn", "full") else []
            )
            if os.environ.get("KERNEL_NPAIRS"):
                attn_pairs = attn_pairs[: int(os.environ["KERNEL_NPAIRS"])]
            DEPTH = 3  # bins in flight ahead of their Y-matmuls (3 so the
            # psY park copies drain before the next (pb, half)'s first Y)
            with (
                tc.tile_pool(name="pts", bufs=DEPTH + 1) as ptpool,
                tc.tile_pool(name="psS", bufs=3, space="PSUM") as psSpool,
                tc.tile_pool(name="psY", bufs=1, space="PSUM") as psYpool,
            ):
                bins = []
                for pb in attn_pairs:
                    for half in range(2):
                        npieces = 0
                        for kb in range(KB if half else 8):
                            q0 = (kb // 2) * P
                            c0 = max(512 * half, q0)
                            c1 = 512 * (half + 1)
                            for h in range(2):
                                if npieces % 2 == 0:
                                    bins.append((pb, half, []))
                                bins[-1][2].append(
                                    (kb, h, c0, c1, q0, 512 * (npieces % 2))
                                )
                                npieces += 1

                psYT = {}

                def emit_y(bin_, pt):
                    pb, half, pieces = bin_
                    lastk = (KB - 1) if half else 7
                    for (kb, h, c0, c1, q0, off) in pieces:
                        key = (pb, half, h)
                        if key not in psYT:
                            psYT[key] = psYpool.tile(
                                [65, 512], f32, tag=f"y{h}", name=f"psYT{h}"
                            )
                        nc.tensor.matmul(
                            psYT[key][:, c0 - 512 * half : c1 - 512 * half],
                            lhsT=VN[:, pb, kb, h],
                            rhs=pt[:, off : off + c1 - c0],
                            start=(kb == 0),
                            stop=(kb == lastk),
                            skip_group_check=True,
                        )
                    if pieces[-1][0] == lastk and pieces[-1][1] == 1:
                        # (pb, half) complete for both heads: park sums +
                        # the unnormalized Y^T, freeing the psum tiles.
                        for hh in range(2):
                            ps = psYT.pop((pb, half, hh))
                            j = 2 * pb + hh
                            nc.vector.tensor_copy(
                                out=SU1[
                                    0:1,
                                    j * 1024
                                    + 512 * half : j * 1024
                                    + 512 * (half + 1),
                                ],
                                in_=ps[64:65, :],
                            )
                            nc.vector.tensor_copy(
                                out=YT[
                                    64 * hh : 64 * hh + 64,
                                    pb,
                                    512 * half : 512 * (half + 1),
                                ],
                                in_=ps[0:64, :],
                            )

                pend = []
                for bi, bin_ in enumerate(bins):
                    pb, half, pieces = bin_
                    binw = max(
                        off + c1 - c0 for (kb, h, c0, c1, q0, off) in pieces
                    )
                    psS = psSpool.tile([P, 1024], f32, tag="s", name="psS")
                    for (kb, h, c0, c1, q0, off) in pieces:
                        nc.tensor.matmul(
                            psS[:, off : off + c1 - c0],
                            lhsT=KT[
                                64 * h : 64 * h + 64, pb, kb * P : (kb + 1) * P
                            ],
                            rhs=QT[64 * h : 64 * h + 64, pb, c0:c1],
                            start=True,
                            stop=True,
                            skip_group_check=True,
                        )
                    for (kb, h, c0, c1, q0, off) in pieces:
                        if c0 <= q0 < c1:
                            mo = off + q0 - c0
                            nc.vector.tensor_add(
                                out=psS[:, mo : mo + P],
                                in0=psS[:, mo : mo + P],
                                in1=msk[:, kb % 2, :],
                            )
                    pt = ptpool.tile([P, 1024], f16, tag="pt", name="pt")
                    nc.scalar.activation(
                        pt[:, :binw], psS[:, :binw], AF.Exp, scale=0.125
                    )
                    pend.append((bin_, pt))
                    if len(pend) > DEPTH:
                        emit_y(*pend.pop(0))
                for item in pend:
                    emit_y(*item)

        if "yt" in dbg_d:
            nc.sync.dma_start(out=dbg_d["yt"], in_=YT)
        if dbg_stage:
            nc.vector.memset(YT[:1, 0, :1], 0.0)

        # batched normalization + output projection, column-half-major so
        # the first four output blocks overlap half 1's normalization.
        # WO loads late into the space freed by XT (its DMA rides out the
        # attention tail).
        out_blocks = list(range(QB)) if dbg_stage in ("", "attn", "full") else []
        with (
            tc.tile_pool(name="wo", bufs=1) as wopool,
            tc.tile_pool(name="z", bufs=2) as zpool,
            tc.tile_pool(name="rs", bufs=3) as rspool,
            tc.tile_pool(name="psN", bufs=2, space="PSUM") as psNpool,
            tc.tile_pool(name="psZ", bufs=2, space="PSUM") as psZpool,
        ):
            WO = wopool.tile([P, 8, C], f16)
            nc.sync.dma_start(out=WO, in_=wout_d)

            def normalize(half):
                # softmax denominators: PE broadcast of the parked sums to 64
                # partitions, DVE fast reciprocal (fp32, ~51 ULP), DVE scale
                # of the unnormalized Y^T.  Keeps ACT out of the tail.
                for pb in (attn_pairs if dbg_stage in ("", "attn", "full") else []):
                    for h in range(2):
                        j = 2 * pb + h
                        rb = psNpool.tile([P, 512], f32, tag="n", name="rb")
                        nc.tensor.matmul(
                            rb[0:64, :],
                            lhsT=on1[:, 0:64],
                            rhs=SU1[
                                0:1,
                                j * 1024
                                + 512 * half : j * 1024
                                + 512 * (half + 1),
                            ],
                            start=True,
                            stop=True,
                        )
                        rsf = rspool.tile([P, 512], f16, tag="rs", name="rsf")
                        rs = rsf[64 * h : 64 * h + 64, :]
                        act_recip(rs, rb[0:64, :])
                        sl = YT[
                            64 * h : 64 * h + 64,
                            pb,
                            512 * half : 512 * (half + 1),
                        ]
                        nc.vector.tensor_mul(out=sl, in0=sl, in1=rs)

            def out_proj(iblocks):
                for i in iblocks:
                    zt = zpool.tile([P, C], f16, tag="z", name="zt")
                    ps = psZpool.tile([P, 1024], f32, tag="z", name="psZ")
                    for nc2 in range(2):
                        sub = ps[:, 512 * nc2 : 512 * (nc2 + 1)]
                        for cb in range(8):
                            nc.tensor.matmul(
                                sub,
                                lhsT=YT[:, cb, i * P : (i + 1) * P],
                                rhs=WO[:, cb, 512 * nc2 : 512 * (nc2 + 1)],
                                start=(cb == 0),
                                stop=(cb == 7),
                                skip_group_check=True,
                            )
                    # bias folded into the psum drain (bo is a host-side
                    # broadcast of b_out to 128 partitions)
                    nc.vector.tensor_add(out=zt, in0=ps, in1=bo)
                    nc.sync.dma_start(out=yr_d[:, i], in_=zt)

            if out_blocks:
                normalize(0)
                out_proj(out_blocks[0:4])
                normalize(1)
                out_proj(out_blocks[4:8])
            else:
                zt = zpool.tile([P, C], f16, tag="z", name="zt")
                nc.vector.memset(zt, 0.0)
                nc.sync.dma_start(out=yr_d[:, 0], in_=zt)

    split_sync_waits(nc)
    return nc


def _host_inputs(x, w_qkv, b_qkv, w_out, b_out):
    x = np.asarray(x, dtype=np.float32)
    w_qkv = np.asarray(w_qkv, dtype=np.float32)
    b_qkv = np.asarray(b_qkv, dtype=np.float32)
    w_out = np.asarray(w_out, dtype=np.float32)
    b_out = np.asarray(b_out, dtype=np.float32)

    wqkv_r0 = np.ascontiguousarray(
        w_qkv[:, :2048].reshape(8, P, 16, P).transpose(2, 1, 0, 3)
    ).astype(np.float16)
    wv_r0 = np.ascontiguousarray(
        w_qkv[:, 2048:].reshape(8, P, 1024).transpose(1, 0, 2)
    ).astype(np.float16)
    wout_r0 = np.ascontiguousarray(
        w_out.reshape(8, P, C).transpose(1, 0, 2)
    ).astype(np.float16)
    bqkv_r0 = np.ascontiguousarray(b_qkv[:2048].reshape(16, P).T)
    bv_r0 = np.ascontiguousarray(
        np.broadcast_to(b_qkv[2048:], (P, 1024))
    ).astype(np.float16)

    def permute_pairs(perm):
        # slot-pair s on the device maps to physical head pair perm[s]: the
        # kernel computes K/V projections for slots 0-3 only and receives
        # slots 4-7 from its partner core, so partners get complementary
        # physical halves.  All per-pair weight/bias layouts permute
        # consistently; the output projection re-contracts over all pairs, so
        # y is permutation-invariant.
        jq = list(perm)
        jk = [8 + p for p in perm]
        wqkv_r = np.ascontiguousarray(wqkv_r0[jq + jk])
        bqkv_r = np.ascontiguousarray(bqkv_r0[:, jq + jk])
        wv_r = np.ascontiguousarray(
            wv_r0.reshape(P, 8, 8, P)[:, :, perm].reshape(P, 8, 1024)
        )
        bv_r = np.ascontiguousarray(
            bv_r0.reshape(P, 8, P)[:, perm].reshape(P, 1024)
        )
        wout_r = np.ascontiguousarray(wout_r0[:, perm])
        return wqkv_r, bqkv_r, wv_r, bv_r, wout_r

    perms = [
        permute_pairs([0, 1, 2, 3, 4, 5, 6, 7]),
        permute_pairs([4, 5, 6, 7, 0, 1, 2, 3]),
    ]
    bout_r = np.ascontiguousarray(
        np.broadcast_to(b_out, (P, C))
    ).astype(np.float16)
    # additive log-masks: 0 = allowed, -3e4 = masked (exp underflows to 0)
    NEG = np.float16(-30000.0)
    tri = np.where(np.triu(np.ones((P, P), dtype=bool)), 0, NEG).astype(
        np.float16
    )  # [k, q]: k <= q allowed
    zer = np.full((P, P), NEG, dtype=np.float16)  # fully masked
    one = np.zeros((P, P), dtype=np.float16)  # fully allowed
    ones1 = np.ones((1, P), dtype=np.float16)

    in_maps = []
    for c in range(N_CORES):
        b, par = c // 2, c % 2
        wqkv_r, bqkv_r, wv_r, bv_r, wout_r = perms[par]
        xb = x[b]
        xT = np.ascontiguousarray(xb.T).astype(np.float16)
        qg = [2 * i + par for i in range(QB)]
        xq = np.concatenate([xb[g * P : (g + 1) * P] for g in qg], axis=0)
        xqT = np.ascontiguousarray(xq.T).astype(np.float16)
        # first-valid-block log-mask by kb parity: the first q block
        # i0 = kb//2 has g0 = 2*(kb//2) + par; g0 == kb -> tri,
        # g0 < kb -> fully masked, g0 > kb -> fully allowed.
        mk = np.empty((P, 2, P), dtype=np.float16)
        mk[:, 0, :] = tri if par == 0 else one
        mk[:, 1, :] = zer if par == 0 else tri
        in_maps.append(
            {
                "xt": xT,
                "xqt": xqT,
                "wqkv": wqkv_r,
                "wv": wv_r,
                "wout": wout_r,
                "bqkv": bqkv_r,
                "bv128": bv_r,
                "bout": bout_r,
                "msk": mk,
                "ones1": ones1,
            }
        )
    return in_maps


def kernel(x, w_qkv, b_qkv, w_out, b_out, trace=False):
    global _COMPILED, LAST_EXEC_NS, LAST_TRACE_PATH
    from concourse import bass_utils

    if _COMPILED is None:
        _COMPILED = _build()
    nc = _COMPILED

    in_maps = _host_inputs(x, w_qkv, b_qkv, w_out, b_out)
    res = bass_utils.run_bass_kernel_spmd(
        nc, in_maps, core_ids=list(range(N_CORES)), trace=trace
    )
    LAST_EXEC_NS = res.exec_time_ns
    if res.instructions_and_trace:
        LAST_TRACE_PATH = res.instructions_and_trace[1]

    y = np.empty((B, T, C), dtype=np.float32)
    for c in range(N_CORES):
        b, par = c // 2, c % 2
        yl = res.results[c]["yr"].transpose(1, 0, 2)  # [QB, P, C]
        for i in range(QB):
            g = 2 * i + par
            y[b, g * P : (g + 1) * P] = yl[i]
    return y



# revision 53
# speedup vs baseline: 1.0476x; 1.0210x over previous
"""Causal self-attention Trainium2 kernel (B=4, T=2048, C=1024, H=16, D=64).

Sharding: 8 cores = 4 batches x 2 causally-balanced query shards.
Core c handles batch b=c//2 and the 8 interleaved query blocks
g = 2*i + (c%2), i in 0..7 (block = 128 rows).  Every core computes full-
sequence K/V for its batch plus Q for its own query rows, runs all 16 heads
of attention for those rows, and the full output projection for them, so
per-core outputs are disjoint row-slices of y: no collectives, no host math.
One program serves both query parities: the per-kb first-block mask input
(tri / zeros / ones) encodes the parity-dependent causal structure.

Device-side dataflow (matmuls fp16 in / fp32 PSUM accumulate).  All matmul
streams are >=~512 columns so LDWEIGHTS hides behind the moving-tensor
stream (a previous 65/128-col structure was weight-load bound):
  Q^T,K^T = w^T @ x^T      (w stationary; [d, t] layout, head pairs stacked)
  V       = x^T.T @ w_v    (x^T chunks stationary, w_v moving -> V in
                            natural [t, c] layout, no PE transpose)
  S^T[kb] = K[kb] @ Q^T    (K block stationary, streams the whole causally
                            valid q-tail; two heads on partition halves)
  P^T     = exp(0.125*S^T) (ACT, psum->sbuf fp16), data mask on the first
                            valid q-block per kb
  Y^T     = [V|1].T @ P^T  (V natural stationary; accumulates [65, 1024]
                            over kb; row 64 = softmax denominators)
  YT      = Y^T * bcast(1/row64)  (PE ones-matmul broadcast + DVE mul)
  Z       = YT.T @ w_out + ones x b_out  (bias as a K=1 matmul)
"""

import os

import numpy as np

B, T, C = 4, 2048, 1024
H, D = 16, 64
N_CORES = 8
P = 128
QB = 8  # local query blocks per core (of 128 rows)
KB = 16  # key blocks per sequence
PAIRS = 8  # head pairs

_COMPILED = None
LAST_EXEC_NS = None
LAST_TRACE_PATH = None

dbg_stage = os.environ.get("KERNEL_DEBUG_STAGE", "")


def _get_mybir():
    import concourse.mybir as mybir
    return mybir


def split_sync_waits(nc):
    mybir = _get_mybir()
    # This walrus build rejects instructions carrying more than one sync
    # wait (or update).  Split the extras onto NOP carriers: waits go on
    # NOPs inserted before the instruction (same engine, so they gate it),
    # updates onto NOPs after it (fire once it has completed).
    uid = [0]

    def carrier(engine, wait=None, update=None):
        uid[0] += 1
        n = mybir.InstNoOp(
            name=f"I-syncsplit-{uid[0]}",
            opcode="NoOp",
            ins=[],
            outs=[],
            sync_info=mybir.SyncInfo(
                on_wait=[wait] if wait else [],
                on_update=[update] if update else [],
            ),
        )
        n.engine = engine
        return n

    for f in nc.m.functions:
        for blk in f.blocks:
            out = []
            changed = False
            for inst in blk.instructions:
                si = inst.sync_info
                if si is None or (
                    len(si.on_wait) <= 1 and len(si.on_update) <= 1
                ):
                    out.append(inst)
                    continue
                changed = True
                waits = list(si.on_wait)
                updates = list(si.on_update)
                for w in waits[1:]:
                    out.append(carrier(inst.engine, wait=w))
                inst.sync_info = mybir.SyncInfo(
                    on_wait=waits[:1], on_update=updates[:1]
                )
                out.append(inst)
                for u in updates[1:]:
                    out.append(carrier(inst.engine, update=u))
            if changed:
                blk.instructions = out


def _build():
    import concourse.bass as bass
    import concourse.tile as tile
    import concourse.mybir as mybir
    from contextlib import ExitStack

    f32 = mybir.dt.float32
    f16 = mybir.dt.float16
    AF = mybir.ActivationFunctionType

    nc = bass.Bass(
        "TRN2", target_bir_lowering=False, debug=False, num_devices=N_CORES
    )

    def act_recip(out, in_):
        # ACT-engine reciprocal (measured max rel err ~9e-4 on [0.5, 3e4],
        # plenty for softmax normalization).  The bass wrapper refuses
        # AF.Reciprocal outright, so emit the InstActivation directly.
        eng = nc.scalar
        inputs = [eng.lower_ap(in_)]
        for arg in (0.0, 1.0, 0.0):  # bias, scale, alpha
            inputs.append(mybir.ImmediateValue(dtype=mybir.dt.float32, value=arg))
        return eng.add_instruction(
            mybir.InstActivation(
                name=nc.get_next_instruction_name(),
                func=AF.Reciprocal,
                ins=inputs,
                outs=[eng.lower_ap(out)],
            )
        )

    xT_d = nc.dram_tensor("xt", [C, T], f16, kind="ExternalInput").ap()
    xqT_d = nc.dram_tensor("xqt", [C, QB * P], f16, kind="ExternalInput").ap()
    wqkv_d = nc.dram_tensor("wqkv", [16, P, 8, P], f16, kind="ExternalInput").ap()
    wv_d = nc.dram_tensor("wv", [P, 8, 1024], f16, kind="ExternalInput").ap()
    wout_d = nc.dram_tensor("wout", [P, 8, C], f16, kind="ExternalInput").ap()
    bqkv_d = nc.dram_tensor("bqkv", [P, 16], f32, kind="ExternalInput").ap()
    bv_d = nc.dram_tensor("bv128", [P, 1024], f16, kind="ExternalInput").ap()
    bout_d = nc.dram_tensor("bout", [P, C], f16, kind="ExternalInput").ap()
    msk_d = nc.dram_tensor("msk", [P, 2, P], f16, kind="ExternalInput").ap()
    ones_d = nc.dram_tensor("ones1", [1, P], f16, kind="ExternalInput").ap()
    yr_d = nc.dram_tensor("yr", [P, QB, C], f16, kind="ExternalOutput").ap()
    dbg_d = {}
    if dbg_stage in ("proj", "vn", "attn"):
        dbg_d["kt"] = nc.dram_tensor(
            "kt", [P, PAIRS, T], f16, kind="ExternalOutput"
        ).ap()
        dbg_d["qt"] = nc.dram_tensor(
            "qt", [P, PAIRS, QB * P], f16, kind="ExternalOutput"
        ).ap()
    if dbg_stage in ("vn", "attn"):
        dbg_d["vn"] = nc.dram_tensor(
            "vn", [P, PAIRS, KB, 2, 65], f16, kind="ExternalOutput"
        ).ap()
    if dbg_stage == "attn":
        dbg_d["yt"] = nc.dram_tensor(
            "yt", [P, 8, QB * P], f16, kind="ExternalOutput"
        ).ap()

    with tile.TileContext(nc) as tc, ExitStack() as ctx:
        persist = ctx.enter_context(tc.tile_pool(name="persist", bufs=1))
        KT = persist.tile([P, PAIRS, T], f16)
        QT = persist.tile([P, PAIRS, QB * P], f16)
        YT = persist.tile([P, 8, QB * P], f16)
        VN = persist.tile([P, PAIRS, KB, 2, 65], f16)
        msk = persist.tile([P, 2, P], f16)
        bqs = persist.tile([P, 16], f32)
        bv = persist.tile([P, 1024], f16)
        on1 = persist.tile([1, P], f16)
        bo = persist.tile([P, C], f16)

        xT_v = xT_d.rearrange("(cb p) t -> p cb t", p=P)
        xqT_v = xqT_d.rearrange("(cb p) t -> p cb t", p=P)
        wq_v = wqkv_d[0:8].rearrange("j p cb q -> p j cb q")
        wk_v = wqkv_d[8:16].rearrange("j p cb q -> p j cb q")

        # overlap mode: K head-pairs 4-7 and the V column half 1 are
        # projected as PE filler inside the (ACT-bound) attention stream
        # of pairs 0-3.  Debug stages need the full projections up front.
        overlap = dbg_stage == ""

        # XT + the pair-0 K weights are hoisted: the K projection runs FIRST
        # so the K-half pair exchange (the long-latency collective) starts as
        # early as possible; V half 0 and Q follow on the PE while the
        # exchange is in flight.
        xtpool = ctx.enter_context(tc.tile_pool(name="xt", bufs=1))
        XT = xtpool.tile([P, 8, T], f16)
        wkp0pool = ctx.enter_context(tc.tile_pool(name="wkp0", bufs=1))
        WKp0 = wkp0pool.tile([P, 1, 8, P], f16)
        nc.sync.dma_start(out=bqs, in_=bqkv_d)
        nc.sync.dma_start(out=WKp0, in_=wk_v[:, 0:1])
        # XT lands in t-quarters: the K projection consumes them in order,
        # so the first matmul starts after 1MB instead of the full 4MB.
        for t4 in range(4):
            nc.sync.dma_start(
                out=XT[:, :, 512 * t4 : 512 * (t4 + 1)],
                in_=xT_v[:, :, 512 * t4 : 512 * (t4 + 1)],
            )
        nc.vector.memset(VN[:, :, :, :, 64:65], 1.0)

        proj_pools = ExitStack()
        wvpool = proj_pools.enter_context(tc.tile_pool(name="wv", bufs=1))
        wkpool = proj_pools.enter_context(tc.tile_pool(name="wk", bufs=1))
        if True:
            WV = wvpool.tile([P, 8, 1024], f16)
            WK = wkpool.tile([P, 3, 8, P], f16)  # K weights, slot pairs 1-3
            nc.sync.dma_start(out=WK, in_=wk_v[:, 1:4])
            nc.sync.dma_start(out=msk, in_=msk_d)
            nc.sync.dma_start(out=bv, in_=bv_d)
            nc.sync.dma_start(out=on1, in_=ones_d)
            nc.sync.dma_start(out=bo, in_=bout_d)
            nc.sync.dma_start(out=WV, in_=wv_d)

            def wk_pb(pb):
                return WKp0[:, 0] if pb == 0 else WK[:, pb - 1]

            with (
                tc.tile_pool(name="psproj", bufs=3, space="PSUM") as pspool,
            ):
                # K^T projection over full T, slot pairs 0-3 (this core's
                # physical half; 4-7 arrive via the pair exchange)
                for pb in range(4):
                    for t4 in range(4):
                        ps = pspool.tile([P, 512], f32, tag="proj")
                        for cb in range(8):
                            nc.tensor.matmul(
                                ps,
                                lhsT=wk_pb(pb)[:, cb],
                                rhs=XT[:, cb, 512 * t4 : 512 * (t4 + 1)],
                                start=(cb == 0),
                                stop=(cb == 7),
                            )
                        nc.scalar.activation(
                            KT[:, pb, 512 * t4 : 512 * (t4 + 1)],
                            ps,
                            AF.Identity,
                            bias=bqs[:, 8 + pb : 9 + pb],
                        )

                # pair-wise K exchange: cores (2b, 2b+1) computed
                # complementary physical head-pair halves (host permutes the
                # weights so each core's slots 0-3 are its own half).  Stage
                # K^T slots 0-3 into both shards of a DRAM bounce;
                # ReduceScatter(add) over the pair gives every rank
                # own+partner at a fixed address; gpsimd subtracts own to
                # recover the partner's half into slots 4-7.  Everything
                # after the collective lives on the gpsimd queue so no
                # compute engine ever waits on collective latency.
                KEX_K = 4 * T
                kexpool = ctx.enter_context(
                    tc.tile_pool(name="kex", bufs=1, space="DRAM")
                )
                kex_in = kexpool.tile([2, P, KEX_K], f16)
                kex_out = kexpool.tile([P, KEX_K], f16)
                for s in range(2):
                    nc.sync.dma_start(
                        out=kex_in[s].rearrange("p (a t) -> p a t", a=4),
                        in_=KT[:, 0:4],
                    )
                nc.gpsimd.collective_compute(
                    "ReduceScatter",
                    mybir.AluOpType.add,
                    replica_groups=[[2 * i, 2 * i + 1] for i in range(4)],
                    ins=[kex_in.opt()],
                    outs=[kex_out.opt()],
                )
                nc.gpsimd.dma_start(
                    out=KT[:, 4:8],
                    in_=kex_out[:].rearrange("p (a t) -> p a t", a=4),
                )
                nc.gpsimd.tensor_sub(
                    out=KT[:, 4:8], in0=KT[:, 4:8], in1=KT[:, 0:4]
                )

                # V natural layout, both column halves (all slot pairs)
                if dbg_stage != "proj":
                    for kb in range(KB):
                        for half in range(2):
                            ps = pspool.tile([P, 512], f32, tag="proj")
                            for cb in range(8):
                                nc.tensor.matmul(
                                    ps,
                                    lhsT=XT[:, cb, kb * P : (kb + 1) * P],
                                    rhs=WV[
                                        :, cb, 512 * half : 512 * (half + 1)
                                    ],
                                    start=(cb == 0),
                                    stop=(cb == 7),
                                )
                            nc.vector.tensor_add(
                                out=VN[
                                    :, 4 * half : 4 * half + 4, kb, :, 0:64
                                ],
                                in0=ps.rearrange(
                                    "p (a b c) -> p a b c", b=2, c=64
                                ),
                                in1=bv[
                                    :, 512 * half : 512 * (half + 1)
                                ].rearrange("p (a b c) -> p a b c", b=2, c=64),
                            )

        # XT/WK/WV free here; Q projection + attention need none of them.
        proj_pools.close()

        supool = ctx.enter_context(tc.tile_pool(name="su", bufs=1))
        SU1 = supool.tile([1, 16 * QB * P], f16)  # softmax sums, part 0

        with (
            tc.tile_pool(name="xqt", bufs=1) as xqtpool,
            tc.tile_pool(name="wq", bufs=1) as wqpool,
            tc.tile_pool(name="psprojq", bufs=3, space="PSUM") as pspool,
        ):
            XQT = xqtpool.tile([P, 8, QB * P], f16)
            WQ = wqpool.tile([P, 8, 8, P], f16)
            nc.sync.dma_start(out=WQ[:, 0:1], in_=wq_v[:, 0:1])
            nc.sync.dma_start(out=XQT[:, 0:8], in_=xqT_v[:, 0:8])
            nc.sync.dma_start(out=WQ[:, 1:8], in_=wq_v[:, 1:8])
            # Q^T projection (j-blocks 0..7): w stationary, XQT moving
            for pb in range(PAIRS):
                for t4 in range(2):
                    ps = pspool.tile([P, 512], f32, tag="proj")
                    for cb in range(8):
                        nc.tensor.matmul(
                            ps,
                            lhsT=WQ[:, pb, cb],
                            rhs=XQT[:, cb, 512 * t4 : 512 * (t4 + 1)],
                            start=(cb == 0),
                            stop=(cb == 7),
                        )
                    nc.scalar.activation(
                        QT[:, pb, 512 * t4 : 512 * (t4 + 1)],
                        ps,
                        AF.Identity,
                        bias=bqs[:, pb : pb + 1],
                    )

        if True:

            if "kt" in dbg_d:
                nc.sync.dma_start(out=dbg_d["kt"], in_=KT)
                nc.sync.dma_start(out=dbg_d["qt"], in_=QT)
            if "vn" in dbg_d:
                nc.sync.dma_start(out=dbg_d["vn"], in_=VN)

            # attention: S^T pieces bin-packed two-per-[P,1024]-psum-tile
            # (one per bank-aligned 512-col compartment; a 64-contract
            # quadrant matmul with start=False hangs the device, so every
            # piece is a bank-aligned start=True).  One ACT exp covers both
            # compartments -- ACT's ~260ns fixed cost per activation
            # dominated the narrow causal tail otherwise.  Causal masking is
            # a pre-exp additive log-mask (0 / -3e4) applied by the DVE on
            # psum, hidden in the ACT queue delay.  Y^T accumulates per
            # column-half in [65, 512] psum tiles; softmax sums (row 64) are
            # parked in SU1.  In overlap mode the K pb4-7 / V half-1
            # projections interleave into pairs 0-3's ACT-bound stream as
            # PE filler.
            attn_pairs = (
                list(range(PAIRS)) if dbg_stage in ("", "attn", "full") else []
            )
            if os.environ.get("KERNEL_NPAIRS"):
                attn_pairs = attn_pairs[: int(os.environ["KERNEL_NPAIRS"])]
            DEPTH = 4  # bins in flight ahead of their Y-matmuls (4 so the
            # psY park copies drain a full bin before the next (pb, half)'s
            # first Y re-allocates the banks)
            with (
                tc.tile_pool(name="pts", bufs=DEPTH + 1) as ptpool,
                tc.tile_pool(name="psS", bufs=3, space="PSUM") as psSpool,
                tc.tile_pool(name="psY", bufs=1, space="PSUM") as psYpool,
            ):
                bins = []
                for pb in attn_pairs:
                    for half in range(2):
                        npieces = 0
                        for kb in range(KB if half else 8):
                            q0 = (kb // 2) * P
                            c0 = max(512 * half, q0)
                            c1 = 512 * (half + 1)
                            for h in range(2):
                                if npieces % 2 == 0:
                                    bins.append((pb, half, []))
                                bins[-1][2].append(
                                    (kb, h, c0, c1, q0, 512 * (npieces % 2))
                                )
                                npieces += 1

                psYT = {}

                def emit_y(bin_, pt):
                    pb, half, pieces = bin_
                    lastk = (KB - 1) if half else 7
                    for (kb, h, c0, c1, q0, off) in pieces:
                        key = (pb, half, h)
                        if key not in psYT:
                            psYT[key] = psYpool.tile(
                                [65, 512], f32, tag=f"y{h}", name=f"psYT{h}"
                            )
                        nc.tensor.matmul(
                            psYT[key][:, c0 - 512 * half : c1 - 512 * half],
                            lhsT=VN[:, pb, kb, h],
                            rhs=pt[:, off : off + c1 - c0],
                            start=(kb == 0),
                            stop=(kb == lastk),
                            skip_group_check=True,
                        )
                    if pieces[-1][0] == lastk and pieces[-1][1] == 1:
                        # (pb, half) complete for both heads: park sums +
                        # the unnormalized Y^T, freeing the psum tiles.
                        for hh in range(2):
                            ps = psYT.pop((pb, half, hh))
                            j = 2 * pb + hh
                            nc.vector.tensor_copy(
                                out=SU1[
                                    0:1,
                                    j * 1024
                                    + 512 * half : j * 1024
                                    + 512 * (half + 1),
                                ],
                                in_=ps[64:65, :],
                            )
                            nc.vector.tensor_copy(
                                out=YT[
                                    64 * hh : 64 * hh + 64,
                                    pb,
                                    512 * half : 512 * (half + 1),
                                ],
                                in_=ps[0:64, :],
                            )

                pend = []
                for bi, bin_ in enumerate(bins):
                    pb, half, pieces = bin_
                    binw = max(
                        off + c1 - c0 for (kb, h, c0, c1, q0, off) in pieces
                    )
                    psS = psSpool.tile([P, 1024], f32, tag="s", name="psS")
                    for (kb, h, c0, c1, q0, off) in pieces:
                        nc.tensor.matmul(
                            psS[:, off : off + c1 - c0],
                            lhsT=KT[
                                64 * h : 64 * h + 64, pb, kb * P : (kb + 1) * P
                            ],
                            rhs=QT[64 * h : 64 * h + 64, pb, c0:c1],
                            start=True,
                            stop=True,
                            skip_group_check=True,
                        )
                    for (kb, h, c0, c1, q0, off) in pieces:
                        if c0 <= q0 < c1:
                            mo = off + q0 - c0
                            nc.vector.tensor_add(
                                out=psS[:, mo : mo + P],
                                in0=psS[:, mo : mo + P],
                                in1=msk[:, kb % 2, :],
                            )
                    pt = ptpool.tile([P, 1024], f16, tag="pt", name="pt")
                    nc.scalar.activation(
                        pt[:, :binw], psS[:, :binw], AF.Exp, scale=0.125
                    )
                    pend.append((bin_, pt))
                    if len(pend) > DEPTH:
                        emit_y(*pend.pop(0))
                for item in pend:
                    emit_y(*item)

        if "yt" in dbg_d:
            nc.sync.dma_start(out=dbg_d["yt"], in_=YT)
        if dbg_stage:
            nc.vector.memset(YT[:1, 0, :1], 0.0)

        # batched normalization + output projection, column-half-major so
        # the first four output blocks overlap half 1's normalization.
        # WO loads late into the space freed by XT (its DMA rides out the
        # attention tail).
        out_blocks = list(range(QB)) if dbg_stage in ("", "attn", "full") else []
        with (
            tc.tile_pool(name="wo", bufs=1) as wopool,
            tc.tile_pool(name="z", bufs=2) as zpool,
            tc.tile_pool(name="rs", bufs=3) as rspool,
            tc.tile_pool(name="psN", bufs=2, space="PSUM") as psNpool,
            tc.tile_pool(name="psZ", bufs=2, space="PSUM") as psZpool,
        ):
            WO = wopool.tile([P, 8, C], f16)
            nc.sync.dma_start(out=WO, in_=wout_d)

            def normalize(half):
                # softmax denominators: PE broadcast of the parked sums to 64
                # partitions, DVE fast reciprocal (fp32, ~51 ULP), DVE scale
                # of the unnormalized Y^T.  Keeps ACT out of the tail.
                for pb in (attn_pairs if dbg_stage in ("", "attn", "full") else []):
                    for h in range(2):
                        j = 2 * pb + h
                        rb = psNpool.tile([P, 512], f32, tag="n", name="rb")
                        nc.tensor.matmul(
                            rb[0:64, :],
                            lhsT=on1[:, 0:64],
                            rhs=SU1[
                                0:1,
                                j * 1024
                                + 512 * half : j * 1024
                                + 512 * (half + 1),
                            ],
                            start=True,
                            stop=True,
                        )
                        rsf = rspool.tile([P, 512], f16, tag="rs", name="rsf")
                        rs = rsf[64 * h : 64 * h + 64, :]
                        act_recip(rs, rb[0:64, :])
                        sl = YT[
                            64 * h : 64 * h + 64,
                            pb,
                            512 * half : 512 * (half + 1),
                        ]
                        nc.vector.tensor_mul(out=sl, in0=sl, in1=rs)

            def out_proj(iblocks):
                for i in iblocks:
                    zt = zpool.tile([P, C], f16, tag="z", name="zt")
                    ps = psZpool.tile([P, 1024], f32, tag="z", name="psZ")
                    for nc2 in range(2):
                        sub = ps[:, 512 * nc2 : 512 * (nc2 + 1)]
                        for cb in range(8):
                            nc.tensor.matmul(
                                sub,
                                lhsT=YT[:, cb, i * P : (i + 1) * P],
                                rhs=WO[:, cb, 512 * nc2 : 512 * (nc2 + 1)],
                                start=(cb == 0),
                                stop=(cb == 7),
                                skip_group_check=True,
                            )
                    # bias folded into the psum drain (bo is a host-side
                    # broadcast of b_out to 128 partitions)
                    nc.vector.tensor_add(out=zt, in0=ps, in1=bo)
                    nc.sync.dma_start(out=yr_d[:, i], in_=zt)

            if out_blocks:
                normalize(0)
                out_proj(out_blocks[0:4])
                normalize(1)
                out_proj(out_blocks[4:8])
            else:
                zt = zpool.tile([P, C], f16, tag="z", name="zt")
                nc.vector.memset(zt, 0.0)
                nc.sync.dma_start(out=yr_d[:, 0], in_=zt)

    split_sync_waits(nc)
    return nc


def _host_inputs(x, w_qkv, b_qkv, w_out, b_out):
    x = np.asarray(x, dtype=np.float32)
    w_qkv = np.asarray(w_qkv, dtype=np.float32)
    b_qkv = np.asarray(b_qkv, dtype=np.float32)
    w_out = np.asarray(w_out, dtype=np.float32)
    b_out = np.asarray(b_out, dtype=np.float32)

    wqkv_r0 = np.ascontiguousarray(
        w_qkv[:, :2048].reshape(8, P, 16, P).transpose(2, 1, 0, 3)
    ).astype(np.float16)
    wv_r0 = np.ascontiguousarray(
        w_qkv[:, 2048:].reshape(8, P, 1024).transpose(1, 0, 2)
    ).astype(np.float16)
    wout_r0 = np.ascontiguousarray(
        w_out.reshape(8, P, C).transpose(1, 0, 2)
    ).astype(np.float16)
    bqkv_r0 = np.ascontiguousarray(b_qkv[:2048].reshape(16, P).T)
    bv_r0 = np.ascontiguousarray(
        np.broadcast_to(b_qkv[2048:], (P, 1024))
    ).astype(np.float16)

    def permute_pairs(perm):
        # slot-pair s on the device maps to physical head pair perm[s]: the
        # kernel computes K/V projections for slots 0-3 only and receives
        # slots 4-7 from its partner core, so partners get complementary
        # physical halves.  All per-pair weight/bias layouts permute
        # consistently; the output projection re-contracts over all pairs, so
        # y is permutation-invariant.
        jq = list(perm)
        jk = [8 + p for p in perm]
        wqkv_r = np.ascontiguousarray(wqkv_r0[jq + jk])
        bqkv_r = np.ascontiguousarray(bqkv_r0[:, jq + jk])
        wv_r = np.ascontiguousarray(
            wv_r0.reshape(P, 8, 8, P)[:, :, perm].reshape(P, 8, 1024)
        )
        bv_r = np.ascontiguousarray(
            bv_r0.reshape(P, 8, P)[:, perm].reshape(P, 1024)
        )
        wout_r = np.ascontiguousarray(wout_r0[:, perm])
        return wqkv_r, bqkv_r, wv_r, bv_r, wout_r

    perms = [
        permute_pairs([0, 1, 2, 3, 4, 5, 6, 7]),
        permute_pairs([4, 5, 6, 7, 0, 1, 2, 3]),
    ]
    bout_r = np.ascontiguousarray(
        np.broadcast_to(b_out, (P, C))
    ).astype(np.float16)
    # additive log-masks: 0 = allowed, -3e4 = masked (exp underflows to 0)
    NEG = np.float16(-30000.0)
    tri = np.where(np.triu(np.ones((P, P), dtype=bool)), 0, NEG).astype(
        np.float16
    )  # [k, q]: k <= q allowed
    zer = np.full((P, P), NEG, dtype=np.float16)  # fully masked
    one = np.zeros((P, P), dtype=np.float16)  # fully allowed
    ones1 = np.ones((1, P), dtype=np.float16)

    in_maps = []
    for c in range(N_CORES):
        b, par = c // 2, c % 2
        wqkv_r, bqkv_r, wv_r, bv_r, wout_r = perms[par]
        xb = x[b]
        xT = np.ascontiguousarray(xb.T).astype(np.float16)
        qg = [2 * i + par for i in range(QB)]
        xq = np.concatenate([xb[g * P : (g + 1) * P] for g in qg], axis=0)
        xqT = np.ascontiguousarray(xq.T).astype(np.float16)
        # first-valid-block log-mask by kb parity: the first q block
        # i0 = kb//2 has g0 = 2*(kb//2) + par; g0 == kb -> tri,
        # g0 < kb -> fully masked, g0 > kb -> fully allowed.
        mk = np.empty((P, 2, P), dtype=np.float16)
        mk[:, 0, :] = tri if par == 0 else one
        mk[:, 1, :] = zer if par == 0 else tri
        in_maps.append(
            {
                "xt": xT,
                "xqt": xqT,
                "wqkv": wqkv_r,
                "wv": wv_r,
                "wout": wout_r,
                "bqkv": bqkv_r,
                "bv128": bv_r,
                "bout": bout_r,
                "msk": mk,
                "ones1": ones1,
            }
        )
    return in_maps


def kernel(x, w_qkv, b_qkv, w_out, b_out, trace=False):
    global _COMPILED, LAST_EXEC_NS, LAST_TRACE_PATH
    from concourse import bass_utils

    if _COMPILED is None:
        _COMPILED = _build()
    nc = _COMPILED

    in_maps = _host_inputs(x, w_qkv, b_qkv, w_out, b_out)
    res = bass_utils.run_bass_kernel_spmd(
        nc, in_maps, core_ids=list(range(N_CORES)), trace=trace
    )
    LAST_EXEC_NS = res.exec_time_ns
    if res.instructions_and_trace:
        LAST_TRACE_PATH = res.instructions_and_trace[1]

    y = np.empty((B, T, C), dtype=np.float32)
    for c in range(N_CORES):
        b, par = c // 2, c % 2
        yl = res.results[c]["yr"].transpose(1, 0, 2)  # [QB, P, C]
        for i in range(QB):
            g = 2 * i + par
            y[b, g * P : (g + 1) * P] = yl[i]
    return y

